# revision 3
# baseline (speedup 1.0000x reference)
"""Trainium2 Bass kernel for DecomposedQValueNN (gnn_message_passing).

Per batch row b of x[65536, 128]:
  xa = x.reshape(B, 32, 4); other_a = MLP_o(xa[:,a]) (3 relu layers, 4-32-32-16)
  sum_other = sum_{a != sel} other_a;  sel_out = MLP_s(xa[:,sel])
  h = relu([sel_out; sum_other] @ gW1 + gb1); q = h @ gW2 + gb2
  out[b] = q[b, clip(int(xa[b,sel,3]),0,1)]

Device mapping (8 cores, pure batch data-parallel, 8192 rows/core):
  activations transposed [feat, batch]; PE 32x32 array tiling runs 16
  per-agent matmuls concurrently; bias+relu fused into PSUM->SBUF
  evacuation alternating between ScalarE and VectorE; masked agent sum via
  ones-matmuls; sel-MLP + global head col-tiled 4 chunks at a time.
  Final 2-way q gather on host.
"""

import numpy as np

B_FULL = 65536
N_CORES = 8
B_C = B_FULL // N_CORES       # 8192
A, D = 32, 4
NCH = 512                     # batch cols per chunk (1 PSUM bank fp32)
CHUNKS = B_C // NCH           # 16
SP = 4                        # chunks per superpass

# wpack column offsets
OW1BD, OW2, OW3, OSW1, OSW2, OSW3 = 0, 256, 288, 320, 352, 384
OONF, OONE, OGSEL, OGSUM, OGW2, OIDN = 416, 448, 480, 512, 544, 576
OB1, OB2, OB3, OSB1, OSB2, OSB3, OGB1, OGB2 = 704, 705, 706, 707, 708, 709, 710, 711
WCOLS = 712

_COMPILED = {}
LAST_RESULT = None


def _f32(a):
    return np.ascontiguousarray(a, dtype=np.float32)


def _build_wpack(sel, oW1, ob1, oW2, ob2, oW3, ob3,
                 sW1, sb1, sW2, sb2, sW3, sb3, gW1, gb1, gW2, gb2):
    P = 128

    def rep4(w):              # [32, m] -> [128, m] per row-group
        return np.tile(_f32(w), (4, 1))

    def repD(w):              # [4, m] -> [128, m] every 4 rows
        return np.tile(_f32(w), (32, 1))

    def padc(w, m):
        w = _f32(w)
        return np.concatenate([w, np.zeros((w.shape[0], m - w.shape[1]), np.float32)], 1)

    # L1 weights as zero-padded K=32 blocks: block u has oW1 at rows 4u..4u+3
    w1bd = np.zeros((32, 8 * 32), np.float32)
    for u in range(8):
        w1bd[4 * u:4 * u + 4, 32 * u:32 * u + 32] = _f32(oW1)
    w1 = np.tile(w1bd, (4, 1))
    w2 = rep4(oW2)
    w3 = rep4(padc(oW3, 32))
    u_sel = sel % 8
    sw1p = np.zeros((32, 32), np.float32)
    sw1p[4 * u_sel:4 * u_sel + 4, :] = _f32(sW1)
    sw1 = np.tile(sw1p, (4, 1))
    sw2 = rep4(sW2)
    sw3 = rep4(padc(sW3, 32))

    # ones matrices for agent sum: out[m] = sum_p ones[p,m] * z3s[p]
    # z3s tile (p,i): agent a = 8i+4p+j at rows 32j..32j+15
    def ones_mat(excl_j):
        o = np.zeros((P, 32), np.float32)
        for j in range(4):
            if j == excl_j:
                continue
            for m in range(16):
                o[32 * j + m, m] = 1.0
        return o

    sel_i, sel_p, sel_j = sel // 8, (sel % 8) // 4, sel % 4
    ones_full = ones_mat(-1)
    ones_excl = ones_mat(sel_j)

    gsel = np.zeros((P, 32), np.float32)
    gsum = np.zeros((P, 32), np.float32)
    for c in range(4):
        gsel[32 * c:32 * c + 16, :] = _f32(gW1)[0:16, :]
        gsum[32 * c:32 * c + 16, :] = _f32(gW1)[16:32, :]
    gw2 = rep4(padc(gW2, 32))
    idn = np.eye(P, dtype=np.float32)

    def bias_col(b, valid=32):
        v = np.zeros((P, 1), np.float32)
        b = _f32(b).ravel()
        for p in range(P):
            r = p % 32
            if r < valid:
                v[p, 0] = b[r % len(b)]
        return v

    parts = [w1, w2, w3, sw1, sw2, sw3, ones_full, ones_excl, gsel, gsum,
             gw2, idn, bias_col(ob1), bias_col(ob2), bias_col(ob3, 16),
             bias_col(sb1), bias_col(sb2), bias_col(sb3, 16),
             bias_col(gb1), bias_col(gb2, 2)]
    wp = np.concatenate(parts, axis=1)
    assert wp.shape == (P, WCOLS), wp.shape
    return wp, (sel_p, sel_i)


def _build_nc(sel_p, sel_i, sel_row, no_tail=False):
    import concourse.bacc as bacc
    import concourse.mybir as mybir
    from concourse.tile import TileContext

    f32 = mybir.dt.float32
    Relu = mybir.ActivationFunctionType.Relu
    Copy = mybir.ActivationFunctionType.Copy
    Ident = mybir.ActivationFunctionType.Identity
    add_op = mybir.AluOpType.add
    max_op = mybir.AluOpType.max

    nc = bacc.Bacc("TRN2", target_bir_lowering=False, debug=False,
                   num_devices=N_CORES)
    x_ext = nc.dram_tensor("x", [B_C, A * D], f32, kind="ExternalInput").ap()
    w_ext = nc.dram_tensor("wpack", [128, WCOLS], f32, kind="ExternalInput").ap()
    o_ext = nc.dram_tensor("out", [2, B_C], f32, kind="ExternalOutput").ap()

    with TileContext(nc) as tc:
        with (
            tc.tile_pool(name="const", bufs=1) as cpool,
            tc.tile_pool(name="xin", bufs=3) as xpool,
            tc.tile_pool(name="xt", bufs=SP + 2) as xtpool,
            tc.tile_pool(name="h", bufs=6) as hpool,
            tc.tile_pool(name="z3s", bufs=12) as z3pool,
            tc.tile_pool(name="gl", bufs=2) as glpool,
            tc.tile_pool(name="zp", bufs=5, space="PSUM") as zpool,
            tc.tile_pool(name="tp", bufs=1, space="PSUM") as tpool,
            tc.tile_pool(name="gp", bufs=2, space="PSUM") as gpool,
        ):
            W = cpool.tile([128, WCOLS], f32, name="W")
            nc.sync.dma_start(out=W[:], in_=w_ext[:])

            def bias(off):
                return W[:, off:off + 1]

            def evac(dst, src, boff, func, dve, lo=0, size=128):
                b = W[lo:lo + size, boff:boff + 1]
                if dve:
                    if func == "relu":
                        nc.vector.tensor_scalar(dst, src, b, 0.0,
                                                add_op, max_op)
                    elif func == "add":
                        nc.vector.tensor_scalar_add(dst, src, b)
                    else:
                        nc.vector.tensor_copy(dst, src)
                else:
                    if func == "relu":
                        nc.scalar.activation(dst, src, Relu, bias=b)
                    elif func == "add":
                        nc.scalar.activation(dst, src, Ident, bias=b)
                    else:
                        nc.scalar.activation(dst, src, Copy)

            for chunk in range(CHUNKS):
                b0 = chunk * NCH
                xin = xpool.tile([128, NCH], f32, tag="xin", name=f"xin{chunk}")
                nc.sync.dma_start(
                    out=xin[:].rearrange("p (k f) -> p k f", f=128),
                    in_=x_ext[b0:b0 + NCH, :].rearrange(
                        "(k p) f -> p k f", p=128))
                tband = tpool.tile([128, NCH], f32, tag="tp", name=f"tband{chunk}")
                for k in range(4):
                    nc.tensor.transpose(
                        tband[:, 128 * k:128 * (k + 1)],
                        xin[:, 128 * k:128 * (k + 1)],
                        W[:, OIDN:OIDN + 128])
                xt = xtpool.tile([128, NCH], f32, tag="xt", name=f"xt{chunk}")
                evac(xt[:], tband[:], 0, "copy", dve=(chunk % 2 == 0))

                z3s_c = [None] * 8
                for p in range(2):
                    banks1 = [zpool.tile([128, NCH], f32, tag="z", name=f"z1_{chunk}_{p}_{i}")
                              for i in range(4)]
                    for i in range(4):
                        for j in range(4):
                            u = 4 * p + j
                            nc.tensor.matmul(
                                banks1[i][32 * j:32 * j + 32, :],
                                W[32 * i:32 * i + 32,
                                  OW1BD + 32 * u:OW1BD + 32 * u + 32],
                                xt[32 * i:32 * i + 32, :],
                                start=True, stop=True,
                                tile_position=(32 * i, 32 * j))
                    h1t = []
                    for i in range(4):
                        h1 = hpool.tile([128, NCH], f32, tag="h1", name=f"h1_{chunk}_{p}_{i}")
                        evac(h1[:], banks1[i][:], OB1, "relu", dve=(i % 2 == 0))
                        h1t.append(h1)
                    banks2 = [zpool.tile([128, NCH], f32, tag="z", name=f"z2_{chunk}_{p}_{i}")
                              for i in range(4)]
                    for i in range(4):
                        for j in range(4):
                            nc.tensor.matmul(
                                banks2[j][32 * i:32 * i + 32, :],
                                W[32 * j:32 * j + 32, OW2:OW2 + 32],
                                h1t[i][32 * j:32 * j + 32, :],
                                start=True, stop=True,
                                tile_position=(32 * j, 32 * i))
                    h2t = []
                    for j in range(4):
                        h2 = hpool.tile([128, NCH], f32, tag="h2", name=f"h2_{chunk}_{p}_{j}")
                        evac(h2[:], banks2[j][:], OB2, "relu", dve=(j % 2 == 1))
                        h2t.append(h2)
                    banks3 = [zpool.tile([128, NCH], f32, tag="z", name=f"z3_{chunk}_{p}_{i}")
                              for i in range(4)]
                    for j in range(4):
                        for i in range(4):
                            nc.tensor.matmul(
                                banks3[i][32 * j:32 * j + 32, :],
                                W[32 * i:32 * i + 32, OW3:OW3 + 32],
                                h2t[j][32 * i:32 * i + 32, :],
                                start=True, stop=True,
                                tile_position=(32 * i, 32 * j))
                    for i in range(4):
                        z3 = z3pool.tile([128, NCH], f32, tag="z3s", name=f"z3s_{chunk}_{p}_{i}")
                        evac(z3[:], banks3[i][:], OB3, "relu", dve=(i % 2 == 0))
                        z3s_c[4 * p + i] = (p, i, z3)

                # ---- per-chunk tail: sel MLP, agent sum, global head ----
                if no_tail:
                    continue
                si = sel_row // 32
                selz1 = gpool.tile([128, NCH], f32, tag="g", name=f"selz1_{chunk}")
                nc.tensor.matmul(
                    selz1[0:32, :],
                    W[32 * si:32 * si + 32, OSW1:OSW1 + 32],
                    xt[32 * si:32 * si + 32, :],
                    start=True, stop=True, tile_position=(32 * si, 0))
                sh1 = glpool.tile([32, NCH], f32, tag="sh1", name=f"sh1_{chunk}")
                evac(sh1[:], selz1[0:32, :], OSB1, "relu", dve=True, size=32)

                selz2 = gpool.tile([128, NCH], f32, tag="g", name=f"selz2_{chunk}")
                nc.tensor.matmul(
                    selz2[0:32, :],
                    W[0:32, OSW2:OSW2 + 32],
                    sh1[0:32, :],
                    start=True, stop=True, tile_position=(0, 0))
                sh2 = glpool.tile([32, NCH], f32, tag="sh2", name=f"sh2_{chunk}")
                evac(sh2[:], selz2[0:32, :], OSB2, "relu", dve=False, size=32)

                selz3 = gpool.tile([128, NCH], f32, tag="g", name=f"selz3_{chunk}")
                nc.tensor.matmul(
                    selz3[0:32, :],
                    W[0:32, OSW3:OSW3 + 32],
                    sh2[0:32, :],
                    start=True, stop=True, tile_position=(0, 0))
                sh3 = glpool.tile([32, NCH], f32, tag="sh3", name=f"sh3_{chunk}")
                evac(sh3[:], selz3[0:32, :], OSB3, "relu", dve=True, size=32)

                # masked agent sum: 8 plain full-array accumulating matmuls
                sumo = gpool.tile([128, NCH], f32, tag="g", name=f"sumo_{chunk}")
                for t in range(8):
                    p, i, z3 = z3s_c[t]
                    oo = OONE if (p == sel_p and i == sel_i) else OONF
                    nc.tensor.matmul(
                        sumo[0:32, :],
                        W[:, oo:oo + 32],
                        z3[:],
                        start=(t == 0), stop=(t == 7))
                sumg = glpool.tile([32, NCH], f32, tag="sumg", name=f"sumg_{chunk}")
                evac(sumg[:], sumo[0:32, :], 0, "copy", dve=(chunk % 2 == 0), size=32)

                # zg = gW1_sel.T @ sel_out + gW1_sum.T @ sum_other (same-bank accum)
                zg = gpool.tile([128, NCH], f32, tag="g", name=f"zg_{chunk}")
                nc.tensor.matmul(
                    zg[0:32, :], W[0:16, OGSEL:OGSEL + 32], sh3[0:16, :],
                    start=True, stop=False)
                nc.tensor.matmul(
                    zg[0:32, :], W[0:16, OGSUM:OGSUM + 32], sumg[0:16, :],
                    start=False, stop=True)
                hg = glpool.tile([32, NCH], f32, tag="hg", name=f"hg_{chunk}")
                evac(hg[:], zg[0:32, :], OGB1, "relu", dve=(chunk % 2 == 1), size=32)

                qp = gpool.tile([128, NCH], f32, tag="g", name=f"qp_{chunk}")
                nc.tensor.matmul(
                    qp[0:32, :], W[0:32, OGW2:OGW2 + 32], hg[0:32, :],
                    start=True, stop=True, tile_position=(0, 0))
                qsb = glpool.tile([32, NCH], f32, tag="q", name=f"qsb_{chunk}")
                evac(qsb[:], qp[0:32, :], OGB2, "add", dve=False, size=32)
                nc.sync.dma_start(out=o_ext[0:1, b0:b0 + NCH], in_=qsb[0:1, :])
                nc.sync.dma_start(out=o_ext[1:2, b0:b0 + NCH], in_=qsb[1:2, :])
    nc.compile()
    return nc


def kernel(**inputs):
    x = _f32(inputs["joint_state_actions"])
    sel = int(inputs["selected_agent_idx"])

    wpack, (sel_p, sel_i) = _build_wpack(
        sel,
        inputs["oW1"], inputs["ob1"], inputs["oW2"], inputs["ob2"],
        inputs["oW3"], inputs["ob3"],
        inputs["sW1"], inputs["sb1"], inputs["sW2"], inputs["sb2"],
        inputs["sW3"], inputs["sb3"],
        inputs["gW1"], inputs["gb1"], inputs["gW2"], inputs["gb2"])

    key = (sel_p, sel_i, sel)
    if key not in _COMPILED:
        _COMPILED[key] = _build_nc(sel_p, sel_i, 4 * sel)
    nc = _COMPILED[key]

    from concourse.bass_utils import run_bass_kernel_spmd
    shards = [np.ascontiguousarray(x[i * B_C:(i + 1) * B_C])
              for i in range(N_CORES)]
    in_maps = [{"x": s, "wpack": wpack} for s in shards]
    import os
    trace = bool(int(os.environ.get("KERNEL_TRACE", "0")))
    res = run_bass_kernel_spmd(nc, in_maps, list(range(N_CORES)),
                               trace=trace)
    global LAST_RESULT
    LAST_RESULT = res

    q01 = np.concatenate([res.results[i]["out"] for i in range(N_CORES)],
                         axis=1)
    act = np.clip(x[:, 4 * sel + 3].astype(np.int32), 0, 1)
    out = np.where(act == 0, q01[0], q01[1]).astype(np.float32)
    return out[:, None]



# revision 4
# speedup vs baseline: 2.0947x; 2.0947x over previous
"""Trainium2 Bass kernel for DecomposedQValueNN (gnn_message_passing).

Per batch row b of x[65536, 128]:
  xa = x.reshape(B, 32, 4); other_a = MLP_o(xa[:,a]) (3 relu layers, 4-32-32-16)
  sum_other = sum_{a != sel} other_a;  sel_out = MLP_s(xa[:,sel])
  h = relu([sel_out; sum_other] @ gW1 + gb1); q = h @ gW2 + gb2
  out[b] = q[b, clip(int(xa[b,sel,3]),0,1)]

V2 design (8 cores, batch data-parallel, 8192 rows/core):
  - host transposes + bf16-casts x to [feat=128, rows] so no on-device
    transpose is needed and DMA reads are contiguous per partition
  - all matmuls bf16 (single-pass on PE vs fp32's 2-pass); PSUM fp32
  - 32x32 PE-array tiling, loop order cycles row groups so LDWEIGHTS
    overlaps in-flight matmuls
  - PSUM pair-tiles [128,1024] (2 banks) so each PSUM->SBUF evacuation
    instruction covers 2 banks; evacs alternate ScalarE/VectorE
  - global head folded into the agent-sum: accumulating matmuls against
    replicated gW1_sum (sel agent excluded via a zeroed weight variant),
    plus one K=16 matmul adding gW1_sel^T @ sel_out
  Final 2-way q gather on host.
"""

import numpy as np
import ml_dtypes

BF16 = ml_dtypes.bfloat16

B_FULL = 65536
N_CORES = 8
B_C = B_FULL // N_CORES       # 8192
A, D = 32, 4
NCH = 512                     # batch cols per PSUM bank (fp32)
CHUNKS = B_C // NCH           # 16

# wpack (bf16) column offsets
OW1BD = 0          # [128, 256] L1 block-diag variants u=0..7
OW2 = 256          # [128, 32]
OW3 = 288          # [128, 32] (cols 16..31 zero)
OSW1 = 320         # [128, 32]
OSW2 = 352
OSW3 = 384
OGSUMF = 416       # [128, 32] replicated gW1[16:32] at 16-row stripes
OGSUME = 448       # same, sel agent's stripe zeroed
OGSEL = 480        # [16, 32] gW1[0:16]
OGW2 = 512         # [32, 32] gW2 padded
WCOLS = 544

_COMPILED = {}
LAST_RESULT = None


def _f32(a):
    return np.ascontiguousarray(a, dtype=np.float32)


def _build_wpack(sel, oW1, oW2, oW3, sW1, sW2, sW3, gW1, gW2):
    P = 128

    def rep4(w):              # [32, m] -> [128, m]
        return np.tile(_f32(w), (4, 1))

    def padc(w, m):
        w = _f32(w)
        return np.concatenate([w, np.zeros((w.shape[0], m - w.shape[1]), np.float32)], 1)

    def padr(w, m):
        w = _f32(w)
        return np.concatenate([w, np.zeros((m - w.shape[0], w.shape[1]), np.float32)], 0)

    # L1 weights as zero-padded K=32 blocks: block u has oW1 at rows 4u..4u+3
    w1bd = np.zeros((32, 8 * 32), np.float32)
    for u in range(8):
        w1bd[4 * u:4 * u + 4, 32 * u:32 * u + 32] = _f32(oW1)
    w1 = np.tile(w1bd, (4, 1))
    w2 = rep4(oW2)
    w3 = rep4(padc(oW3, 32))
    u_sel = sel % 8
    sw1p = np.zeros((32, 32), np.float32)
    sw1p[4 * u_sel:4 * u_sel + 4, :] = _f32(sW1)
    sw1 = np.tile(sw1p, (4, 1))
    sw2 = rep4(_f32(sW2))
    sw3 = rep4(padc(sW3, 32))

    # z3s layout: agent (8i + 4p + j) z3[m] at partition 32j+m of pair-tile
    # i//2, col-half (i%2).  gsum weight: row 32j+m -> gW1[16+m, :].
    g = _f32(gW1)
    gsumf = np.zeros((P, 32), np.float32)
    for j in range(4):
        gsumf[32 * j:32 * j + 16, :] = g[16:32, :]
    gsume = gsumf.copy()
    j_sel = sel % 4
    gsume[32 * j_sel:32 * j_sel + 16, :] = 0.0

    gselw = padr(g[0:16, :], P)          # used as wp[0:16, OGSEL:]
    gw2 = padr(padc(gW2, 32), P)         # wp[0:32, OGW2:]

    parts = [w1, w2, w3, sw1, sw2, sw3, gsumf, gsume, gselw, gw2]
    wp = np.concatenate(parts, axis=1)
    assert wp.shape == (P, WCOLS), wp.shape
    return np.ascontiguousarray(wp.astype(BF16))


def _build_bias(ob1, ob2, ob3, sb1, sb2, sb3, gb1, gb2):
    # fp32 per-partition bias columns: col k pattern = b[(p % 32) % len]
    P = 128

    def bias_col(b, valid=32):
        v = np.zeros((P, 1), np.float32)
        b = _f32(b).ravel()
        for p in range(P):
            r = p % 32
            if r < valid:
                v[p, 0] = b[r % len(b)]
        return v

    cols = [bias_col(ob1), bias_col(ob2), bias_col(ob3, 16),
            bias_col(sb1), bias_col(sb2), bias_col(sb3, 16),
            bias_col(gb1), bias_col(gb2, 2)]
    return np.ascontiguousarray(np.concatenate(cols, 1))  # [128, 8] fp32


BB1, BB2, BB3, BSB1, BSB2, BSB3, BGB1, BGB2 = range(8)


def _build_nc(sel):
    import concourse.bacc as bacc
    import concourse.mybir as mybir
    from concourse.tile import TileContext

    f32 = mybir.dt.float32
    bf16 = mybir.dt.bfloat16
    Relu = mybir.ActivationFunctionType.Relu
    Ident = mybir.ActivationFunctionType.Identity
    add_op = mybir.AluOpType.add
    max_op = mybir.AluOpType.max

    i_sel, j_sel = sel // 8, sel % 4
    p_sel = (sel % 8) // 4
    si = sel // 8          # partition group of sel agent's features

    nc = bacc.Bacc("TRN2", target_bir_lowering=False, debug=False,
                   num_devices=N_CORES)
    x_ext = nc.dram_tensor("xt", [128, B_C], bf16, kind="ExternalInput").ap()
    w_ext = nc.dram_tensor("wpack", [128, WCOLS], bf16, kind="ExternalInput").ap()
    b_ext = nc.dram_tensor("bias", [128, 8], f32, kind="ExternalInput").ap()
    o_ext = nc.dram_tensor("out", [2, B_C], f32, kind="ExternalOutput").ap()

    with TileContext(nc) as tc:
        with (
            tc.tile_pool(name="const", bufs=1) as cpool,
            tc.tile_pool(name="xin", bufs=3) as xpool,
            tc.tile_pool(name="h", bufs=5) as hpool,
            tc.tile_pool(name="z3s", bufs=5) as z3pool,
            tc.tile_pool(name="gl", bufs=4) as glpool,
            tc.tile_pool(name="osb", bufs=1) as opool,
            tc.tile_pool(name="zp", bufs=3, space="PSUM") as zpool,
            tc.tile_pool(name="gp", bufs=2, space="PSUM") as gpool,
        ):
            W = cpool.tile([128, WCOLS], bf16, name="W")
            nc.sync.dma_start(out=W[:], in_=w_ext[:])
            BI = cpool.tile([128, 8], f32, name="BI")
            nc.sync.dma_start(out=BI[:], in_=b_ext[:])
            outsb = opool.tile([2, B_C], f32, name="outsb")

            def evac(dst, src, bcol, func, dve, lo=0, size=128):
                b = BI[lo:lo + size, bcol:bcol + 1]
                if dve:
                    if func == "relu":
                        nc.vector.tensor_scalar(dst, src, b, 0.0, add_op, max_op)
                    else:
                        nc.vector.tensor_scalar_add(dst, src, b)
                else:
                    if func == "relu":
                        nc.scalar.activation(dst, src, Relu, bias=b)
                    else:
                        nc.scalar.activation(dst, src, Ident, bias=b)

            for chunk in range(CHUNKS):
                b0 = chunk * NCH
                xin = xpool.tile([128, NCH], bf16, tag="xin", name=f"xin{chunk}")
                nc.sync.dma_start(out=xin[:], in_=x_ext[:, b0:b0 + NCH])

                z3s_t = {}
                h1_t = {}
                h2_t = {}
                for p in range(2):
                    # ---- L1: 16 tile-MMs -> 2 pair-tiles (banks by i) ----
                    z1 = [zpool.tile([128, 2 * NCH], f32, tag="z",
                                     name=f"z1_{chunk}_{p}_{k}") for k in range(2)]
                    for j in range(4):
                        for i in range(4):
                            u = 4 * p + j
                            nc.tensor.matmul(
                                z1[i // 2][32 * j:32 * j + 32,
                                           NCH * (i % 2):NCH * (i % 2) + NCH],
                                W[32 * i:32 * i + 32,
                                  OW1BD + 32 * u:OW1BD + 32 * u + 32],
                                xin[32 * i:32 * i + 32, :],
                                start=True, stop=True,
                                tile_position=(32 * i, 32 * j))
                    for k in range(2):
                        h1 = hpool.tile([128, 2 * NCH], bf16, tag="h1",
                                        name=f"h1_{chunk}_{p}_{k}")
                        evac(h1[:], z1[k][:], BB1, "relu", dve=(k == 0))
                        h1_t[(p, k)] = h1

                    # ---- L2: agent 8i+4p+j: h1 at h1_t[(p,i//2)][32j, half i%2]
                    z2 = [zpool.tile([128, 2 * NCH], f32, tag="z",
                                     name=f"z2_{chunk}_{p}_{k}") for k in range(2)]
                    for i in range(4):
                        for j in range(4):
                            nc.tensor.matmul(
                                z2[j // 2][32 * i:32 * i + 32,
                                           NCH * (j % 2):NCH * (j % 2) + NCH],
                                W[32 * j:32 * j + 32, OW2:OW2 + 32],
                                h1_t[(p, i // 2)][32 * j:32 * j + 32,
                                                  NCH * (i % 2):NCH * (i % 2) + NCH],
                                start=True, stop=True,
                                tile_position=(32 * j, 32 * i))
                    for k in range(2):
                        h2 = hpool.tile([128, 2 * NCH], bf16, tag="h2",
                                        name=f"h2_{chunk}_{p}_{k}")
                        evac(h2[:], z2[k][:], BB2, "relu", dve=(k == 1))
                        h2_t[(p, k)] = h2

                    # ---- L3: agent 8i+4p+j: h2 at h2_t[(p,j//2)][32i, half j%2]
                    # out z3[m] -> pair i//2, partitions 32j+m, half i%2
                    z3 = [zpool.tile([128, 2 * NCH], f32, tag="z",
                                     name=f"z3_{chunk}_{p}_{k}") for k in range(2)]
                    for j in range(4):
                        for i in range(4):
                            nc.tensor.matmul(
                                z3[i // 2][32 * j:32 * j + 32,
                                           NCH * (i % 2):NCH * (i % 2) + NCH],
                                W[32 * i:32 * i + 32, OW3:OW3 + 32],
                                h2_t[(p, j // 2)][32 * i:32 * i + 32,
                                                  NCH * (j % 2):NCH * (j % 2) + NCH],
                                start=True, stop=True,
                                tile_position=(32 * i, 32 * j))
                    for k in range(2):
                        z3sb = z3pool.tile([128, 2 * NCH], bf16, tag="z3s",
                                           name=f"z3s_{chunk}_{p}_{k}")
                        evac(z3sb[:], z3[k][:], BB3, "relu", dve=(k == 0))
                        z3s_t[(p, k)] = z3sb

                # ---- sel MLP (bf16, per-chunk serial) ----
                selz1 = gpool.tile([32, NCH], f32, tag="g", name=f"selz1_{chunk}")
                nc.tensor.matmul(
                    selz1[:], W[32 * si:32 * si + 32, OSW1:OSW1 + 32],
                    xin[32 * si:32 * si + 32, :],
                    start=True, stop=True, tile_position=(32 * si, 0))
                sh1 = glpool.tile([32, NCH], bf16, tag="sh1", name=f"sh1_{chunk}")
                evac(sh1[:], selz1[:], BSB1, "relu", dve=True, size=32)

                selz2 = gpool.tile([32, NCH], f32, tag="g", name=f"selz2_{chunk}")
                nc.tensor.matmul(
                    selz2[:], W[0:32, OSW2:OSW2 + 32], sh1[0:32, :],
                    start=True, stop=True, tile_position=(0, 0))
                sh2 = glpool.tile([32, NCH], bf16, tag="sh2", name=f"sh2_{chunk}")
                evac(sh2[:], selz2[:], BSB2, "relu", dve=False, size=32)

                selz3 = gpool.tile([32, NCH], f32, tag="g", name=f"selz3_{chunk}")
                nc.tensor.matmul(
                    selz3[:], W[0:32, OSW3:OSW3 + 32], sh2[0:32, :],
                    start=True, stop=True, tile_position=(0, 0))
                sh3 = glpool.tile([32, NCH], bf16, tag="sh3", name=f"sh3_{chunk}")
                evac(sh3[:], selz3[:], BSB3, "relu", dve=True, size=32)

                # ---- fused global head: zg = sum_a!=sel gW1s^T z3_a + gWsel^T sh3
                zg = gpool.tile([32, NCH], f32, tag="g", name=f"zg_{chunk}")
                n_mm = 0
                for p in range(2):
                    for k in range(2):
                        for h in range(2):
                            i = 2 * k + h
                            excl = (p == p_sel and i == i_sel)
                            oo = OGSUME if excl else OGSUMF
                            nc.tensor.matmul(
                                zg[:], W[:, oo:oo + 32],
                                z3s_t[(p, k)][:, NCH * h:NCH * h + NCH],
                                start=(n_mm == 0), stop=False)
                            n_mm += 1
                nc.tensor.matmul(
                    zg[:], W[0:16, OGSEL:OGSEL + 32], sh3[0:16, :],
                    start=False, stop=True)
                hg = glpool.tile([32, NCH], bf16, tag="hg", name=f"hg_{chunk}")
                evac(hg[:], zg[:], BGB1, "relu", dve=(chunk % 2 == 0), size=32)

                qp = gpool.tile([32, NCH], f32, tag="g", name=f"qp_{chunk}")
                nc.tensor.matmul(
                    qp[:], W[0:32, OGW2:OGW2 + 32], hg[0:32, :],
                    start=True, stop=True, tile_position=(0, 0))
                evac(outsb[0:2, b0:b0 + NCH], qp[0:2, :], BGB2, "add",
                     dve=(chunk % 2 == 1), size=2)

            nc.sync.dma_start(out=o_ext[:], in_=outsb[:])
    nc.compile()
    return nc


def kernel(**inputs):
    x = _f32(inputs["joint_state_actions"])
    sel = int(inputs["selected_agent_idx"])

    wpack = _build_wpack(
        sel, inputs["oW1"], inputs["oW2"], inputs["oW3"],
        inputs["sW1"], inputs["sW2"], inputs["sW3"],
        inputs["gW1"], inputs["gW2"])
    bias = _build_bias(
        inputs["ob1"], inputs["ob2"], inputs["ob3"],
        inputs["sb1"], inputs["sb2"], inputs["sb3"],
        inputs["gb1"], inputs["gb2"])

    if sel not in _COMPILED:
        _COMPILED[sel] = _build_nc(sel)
    nc = _COMPILED[sel]

    from concourse.bass_utils import run_bass_kernel_spmd
    shards = [np.ascontiguousarray(x[i * B_C:(i + 1) * B_C].T.astype(BF16))
              for i in range(N_CORES)]
    in_maps = [{"xt": s, "wpack": wpack, "bias": bias} for s in shards]
    import os
    trace = bool(int(os.environ.get("KERNEL_TRACE", "0")))
    res = run_bass_kernel_spmd(nc, in_maps, list(range(N_CORES)),
                               trace=trace)
    global LAST_RESULT
    LAST_RESULT = res

    q01 = np.concatenate([res.results[i]["out"] for i in range(N_CORES)],
                         axis=1)
    act = np.clip(x[:, 4 * sel + 3].astype(np.int32), 0, 1)
    out = np.where(act == 0, q01[0], q01[1]).astype(np.float32)
    return out[:, None]


# revision 11
# speedup vs baseline: 2.4483x; 1.1688x over previous
"""Trainium2 Bass kernel for DecomposedQValueNN (gnn_message_passing).

Per batch row b of x[65536, 128]:
  xa = x.reshape(B, 32, 4); other_a = MLP_o(xa[:,a]) (3 relu layers, 4-32-32-16)
  sum_other = sum_{a != sel} other_a;  sel_out = MLP_s(xa[:,sel])
  h = relu([sel_out; sum_other] @ gW1 + gb1); q = h @ gW2 + gb2
  out[b] = q[b, clip(int(xa[b,sel,3]),0,1)]

V2 design (8 cores, batch data-parallel, 8192 rows/core):
  - host transposes + bf16-casts x to [feat=128, rows] so no on-device
    transpose is needed and DMA reads are contiguous per partition
  - all matmuls bf16 (single-pass on PE vs fp32's 2-pass); PSUM fp32
  - 32x32 PE-array tiling, loop order cycles row groups so LDWEIGHTS
    overlaps in-flight matmuls
  - PSUM pair-tiles [128,1024] (2 banks) so each PSUM->SBUF evacuation
    instruction covers 2 banks; evacs alternate ScalarE/VectorE
  - global head folded into the agent-sum: accumulating matmuls against
    replicated gW1_sum (sel agent excluded via a zeroed weight variant),
    plus one K=16 matmul adding gW1_sel^T @ sel_out
  Final 2-way q gather on host.
"""

import numpy as np
import ml_dtypes

BF16 = ml_dtypes.bfloat16

B_FULL = 65536
N_CORES = 8
B_C = B_FULL // N_CORES       # 8192
A, D = 32, 4
NCH = 512                     # batch cols per PSUM bank (fp32)
CHUNKS = B_C // NCH           # 16

# wpack (bf16) column offsets
OW1BD = 0          # [128, 256] L1 block-diag variants u=0..7
OW2 = 256          # [128, 32]
OW3P = 288         # [128, 32] K=64 2-agent pack: even row-grp W3->cols 0..15,
                   # odd row-grp W3->cols 16..31
OSW1 = 320         # [128, 32]
OSW2 = 352
OSW3 = 384
OGSUMF = 416       # [128, 32] gW1[16+(m%16)] at every partition 32j+m
OGSUME = 448       # same, sel agent's 16-row stripe zeroed
OGSEL = 480        # [16, 32] gW1[0:16]
OGW2 = 512         # [32, 32] gW2 padded
WCOLS = 544

_COMPILED = {}
LAST_RESULT = None


def _f32(a):
    return np.ascontiguousarray(a, dtype=np.float32)


def _build_wpack(sel, oW1, oW2, oW3, sW1, sW2, sW3, gW1, gW2):
    P = 128

    def rep4(w):              # [32, m] -> [128, m]
        return np.tile(_f32(w), (4, 1))

    def padc(w, m):
        w = _f32(w)
        return np.concatenate([w, np.zeros((w.shape[0], m - w.shape[1]), np.float32)], 1)

    def padr(w, m):
        w = _f32(w)
        return np.concatenate([w, np.zeros((m - w.shape[0], w.shape[1]), np.float32)], 0)

    # L1 weights as zero-padded K=32 blocks: block u has oW1 at rows 4u..4u+3
    w1bd = np.zeros((32, 8 * 32), np.float32)
    for u in range(8):
        w1bd[4 * u:4 * u + 4, 32 * u:32 * u + 32] = _f32(oW1)
    w1 = np.tile(w1bd, (4, 1))
    w2 = rep4(oW2)
    # K=64 2-agent pack: row-group r even -> z3 cols 0..15, odd -> 16..31
    w3L = padc(_f32(oW3), 32)                       # [32, 32] cols 0..15
    w3R = np.concatenate([np.zeros((32, 16), np.float32), _f32(oW3)], 1)
    w3 = np.concatenate([w3L, w3R, w3L, w3R], 0)    # [128, 32]
    u_sel = sel % 8
    sw1p = np.zeros((32, 32), np.float32)
    sw1p[4 * u_sel:4 * u_sel + 4, :] = _f32(sW1)
    sw1 = np.tile(sw1p, (4, 1))
    sw2 = rep4(_f32(sW2))
    sw3 = rep4(padc(sW3, 32))

    # z3s layout (K=64 packed): per p one pair-tile; half k=i1//2 holds
    # agents (8*i1+4p+j) at partitions 32j+0..15 and (8*(i1+1)+4p+j) at
    # 32j+16..31.  gsum weight: row 32j+m -> gW1[16+(m%16), :].
    g = _f32(gW1)
    gsumf = np.zeros((P, 32), np.float32)
    for j in range(4):
        gsumf[32 * j:32 * j + 16, :] = g[16:32, :]
        gsumf[32 * j + 16:32 * j + 32, :] = g[16:32, :]
    gsume = gsumf.copy()
    j_sel = sel % 4
    i_sel = sel // 8
    off = 32 * j_sel + 16 * (i_sel % 2)
    gsume[off:off + 16, :] = 0.0

    gselw = padr(g[0:16, :], P)          # used as wp[0:16, OGSEL:]
    gw2 = padr(padc(gW2, 32), P)         # wp[0:32, OGW2:]

    parts = [w1, w2, w3, sw1, sw2, sw3, gsumf, gsume, gselw, gw2]
    wp = np.concatenate(parts, axis=1)
    assert wp.shape == (P, WCOLS), wp.shape
    return np.ascontiguousarray(wp.astype(BF16))


def _build_bias(ob1, ob2, ob3, sb1, sb2, sb3, gb1, gb2):
    # fp32 per-partition bias columns: col k pattern = b[(p % 32) % len]
    P = 128

    def bias_col(b, valid=32):
        v = np.zeros((P, 1), np.float32)
        b = _f32(b).ravel()
        for p in range(P):
            r = p % 32
            if r < valid:
                v[p, 0] = b[r % len(b)]
        return v

    cols = [bias_col(ob1), bias_col(ob2), bias_col(ob3, 32),
            bias_col(sb1), bias_col(sb2), bias_col(sb3, 16),
            bias_col(gb1), bias_col(gb2, 2)]
    return np.ascontiguousarray(np.concatenate(cols, 1))  # [128, 8] fp32


BB1, BB2, BB3, BSB1, BSB2, BSB3, BGB1, BGB2 = range(8)


def _build_nc(sel):
    import concourse.bacc as bacc
    import concourse.mybir as mybir
    from concourse.tile import TileContext

    f32 = mybir.dt.float32
    bf16 = mybir.dt.bfloat16
    Relu = mybir.ActivationFunctionType.Relu
    Ident = mybir.ActivationFunctionType.Identity
    add_op = mybir.AluOpType.add
    max_op = mybir.AluOpType.max

    i_sel, j_sel = sel // 8, sel % 4
    p_sel = (sel % 8) // 4
    si = sel // 8          # partition group of sel agent's features

    nc = bacc.Bacc("TRN2", target_bir_lowering=False, debug=False,
                   num_devices=N_CORES)
    x_ext = nc.dram_tensor("xt", [128, B_C], bf16, kind="ExternalInput").ap()
    w_ext = nc.dram_tensor("wpack", [128, WCOLS], bf16, kind="ExternalInput").ap()
    b_ext = nc.dram_tensor("bias", [128, 8], f32, kind="ExternalInput").ap()
    o_ext = nc.dram_tensor("out", [2, B_C], f32, kind="ExternalOutput").ap()

    with TileContext(nc) as tc:
        with (
            tc.tile_pool(name="const", bufs=1) as cpool,
            tc.tile_pool(name="xin", bufs=3) as xpool,
            tc.tile_pool(name="h", bufs=5) as hpool,
            tc.tile_pool(name="z3s", bufs=5) as z3pool,
            tc.tile_pool(name="gl", bufs=4) as glpool,
            tc.tile_pool(name="osb", bufs=1) as opool,
            tc.tile_pool(name="zp", bufs=3, space="PSUM") as zpool,
            tc.tile_pool(name="gp", bufs=2, space="PSUM") as gpool,
        ):
            W = cpool.tile([128, WCOLS], bf16, name="W")
            nc.sync.dma_start(out=W[:], in_=w_ext[:])
            BI = cpool.tile([128, 8], f32, name="BI")
            nc.sync.dma_start(out=BI[:], in_=b_ext[:])
            outsb = opool.tile([2, B_C], f32, name="outsb")

            def evac(dst, src, bcol, func, dve, lo=0, size=128):
                b = BI[lo:lo + size, bcol:bcol + 1]
                if dve:
                    if func == "relu":
                        nc.vector.tensor_scalar(dst, src, b, 0.0, add_op, max_op)
                    else:
                        nc.vector.tensor_scalar_add(dst, src, b)
                else:
                    if func == "relu":
                        nc.scalar.activation(dst, src, Relu, bias=b)
                    else:
                        nc.scalar.activation(dst, src, Ident, bias=b)

            for chunk in range(CHUNKS):
                b0 = chunk * NCH
                xin = xpool.tile([128, NCH], bf16, tag="xin", name=f"xin{chunk}")
                nc.sync.dma_start(out=xin[:], in_=x_ext[:, b0:b0 + NCH])

                z3s_t = {}
                h1_t = {}
                h2_t = {}
                for p in range(2):
                    # ---- L1: 16 tile-MMs -> 2 pair-tiles (banks by i) ----
                    z1 = [zpool.tile([128, 2 * NCH], f32, tag="z",
                                     name=f"z1_{chunk}_{p}_{k}") for k in range(2)]
                    for j in range(4):
                        for i in range(4):
                            u = 4 * p + j
                            nc.tensor.matmul(
                                z1[i // 2][32 * j:32 * j + 32,
                                           NCH * (i % 2):NCH * (i % 2) + NCH],
                                W[32 * i:32 * i + 32,
                                  OW1BD + 32 * u:OW1BD + 32 * u + 32],
                                xin[32 * i:32 * i + 32, :],
                                start=True, stop=True,
                                tile_position=(32 * i, 32 * j))
                    for k in range(2):
                        h1 = hpool.tile([128, 2 * NCH], bf16, tag="h1",
                                        name=f"h1_{chunk}_{p}_{k}")
                        evac(h1[:], z1[k][:], BB1, "relu", dve=(k == 0))
                        h1_t[(p, k)] = h1

                    # ---- L2: agent 8i+4p+j: h1 at h1_t[(p,i//2)][32j, half i%2]
                    z2 = [zpool.tile([128, 2 * NCH], f32, tag="z",
                                     name=f"z2_{chunk}_{p}_{k}") for k in range(2)]
                    for i in range(4):
                        for j in range(4):
                            nc.tensor.matmul(
                                z2[j // 2][32 * i:32 * i + 32,
                                           NCH * (j % 2):NCH * (j % 2) + NCH],
                                W[32 * j:32 * j + 32, OW2:OW2 + 32],
                                h1_t[(p, i // 2)][32 * j:32 * j + 32,
                                                  NCH * (i % 2):NCH * (i % 2) + NCH],
                                start=True, stop=True,
                                tile_position=(32 * j, 32 * i))
                    for k in range(2):
                        h2 = hpool.tile([128, 2 * NCH], bf16, tag="h2",
                                        name=f"h2_{chunk}_{p}_{k}")
                        evac(h2[:], z2[k][:], BB2, "relu", dve=(k == 0))
                        h2_t[(p, k)] = h2

                    # ---- L3 (K=64, 2 agents/tile): agents 8*i1+4p+j and
                    # 8*(i1+1)+4p+j from h2_t[(p,j//2)][32*i1:32*i1+64];
                    # out -> partitions 32j (16+16), col-half i1//2
                    z3 = zpool.tile([128, 2 * NCH], f32, tag="z",
                                    name=f"z3_{chunk}_{p}")
                    for j in range(4):
                        for i1 in (0, 2):
                            nc.tensor.matmul(
                                z3[32 * j:32 * j + 32,
                                   NCH * (i1 // 2):NCH * (i1 // 2) + NCH],
                                W[32 * i1:32 * i1 + 64, OW3P:OW3P + 32],
                                h2_t[(p, j // 2)][32 * i1:32 * i1 + 64,
                                                  NCH * (j % 2):NCH * (j % 2) + NCH],
                                start=True, stop=True,
                                tile_position=(32 * i1, 32 * j))
                    z3sb = z3pool.tile([128, 2 * NCH], bf16, tag="z3s",
                                       name=f"z3s_{chunk}_{p}")
                    evac(z3sb[:], z3[:], BB3, "relu", dve=False)
                    z3s_t[p] = z3sb

                # ---- sel MLP (bf16, per-chunk serial) ----
                selz1 = gpool.tile([32, NCH], f32, tag="g", name=f"selz1_{chunk}")
                nc.tensor.matmul(
                    selz1[:], W[32 * si:32 * si + 32, OSW1:OSW1 + 32],
                    xin[32 * si:32 * si + 32, :],
                    start=True, stop=True, tile_position=(32 * si, 0))
                sh1 = glpool.tile([32, NCH], bf16, tag="sh1", name=f"sh1_{chunk}")
                evac(sh1[:], selz1[:], BSB1, "relu", dve=True, size=32)

                selz2 = gpool.tile([32, NCH], f32, tag="g", name=f"selz2_{chunk}")
                nc.tensor.matmul(
                    selz2[:], W[0:32, OSW2:OSW2 + 32], sh1[0:32, :],
                    start=True, stop=True, tile_position=(0, 0))
                sh2 = glpool.tile([32, NCH], bf16, tag="sh2", name=f"sh2_{chunk}")
                evac(sh2[:], selz2[:], BSB2, "relu", dve=True, size=32)

                selz3 = gpool.tile([32, NCH], f32, tag="g", name=f"selz3_{chunk}")
                nc.tensor.matmul(
                    selz3[:], W[0:32, OSW3:OSW3 + 32], sh2[0:32, :],
                    start=True, stop=True, tile_position=(0, 0))
                sh3 = glpool.tile([32, NCH], bf16, tag="sh3", name=f"sh3_{chunk}")
                evac(sh3[:], selz3[:], BSB3, "relu", dve=True, size=32)

                # ---- fused global head: zg = sum_a!=sel gW1s^T z3_a + gWsel^T sh3
                zg = gpool.tile([32, NCH], f32, tag="g", name=f"zg_{chunk}")
                n_mm = 0
                for p in range(2):
                    for k in range(2):
                        excl = (p == p_sel and k == i_sel // 2)
                        oo = OGSUME if excl else OGSUMF
                        nc.tensor.matmul(
                            zg[:], W[:, oo:oo + 32],
                            z3s_t[p][:, NCH * k:NCH * k + NCH],
                            start=(n_mm == 0), stop=False)
                        n_mm += 1
                nc.tensor.matmul(
                    zg[:], W[0:16, OGSEL:OGSEL + 32], sh3[0:16, :],
                    start=False, stop=True)
                hg = glpool.tile([32, NCH], bf16, tag="hg", name=f"hg_{chunk}")
                evac(hg[:], zg[:], BGB1, "relu", dve=True, size=32)

                qp = gpool.tile([32, NCH], f32, tag="g", name=f"qp_{chunk}")
                nc.tensor.matmul(
                    qp[:], W[0:32, OGW2:OGW2 + 32], hg[0:32, :],
                    start=True, stop=True, tile_position=(0, 0))
                evac(outsb[0:2, b0:b0 + NCH], qp[0:2, :], BGB2, "add",
                     dve=False, size=2)

            nc.sync.dma_start(out=o_ext[:], in_=outsb[:])
    nc.compile()
    return nc


def kernel(**inputs):
    x = _f32(inputs["joint_state_actions"])
    sel = int(inputs["selected_agent_idx"])

    wpack = _build_wpack(
        sel, inputs["oW1"], inputs["oW2"], inputs["oW3"],
        inputs["sW1"], inputs["sW2"], inputs["sW3"],
        inputs["gW1"], inputs["gW2"])
    bias = _build_bias(
        inputs["ob1"], inputs["ob2"], inputs["ob3"],
        inputs["sb1"], inputs["sb2"], inputs["sb3"],
        inputs["gb1"], inputs["gb2"])

    if sel not in _COMPILED:
        _COMPILED[sel] = _build_nc(sel)
    nc = _COMPILED[sel]

    from concourse.bass_utils import run_bass_kernel_spmd
    shards = [np.ascontiguousarray(x[i * B_C:(i + 1) * B_C].T.astype(BF16))
              for i in range(N_CORES)]
    in_maps = [{"xt": s, "wpack": wpack, "bias": bias} for s in shards]
    import os
    trace = bool(int(os.environ.get("KERNEL_TRACE", "0")))
    res = run_bass_kernel_spmd(nc, in_maps, list(range(N_CORES)),
                               trace=trace)
    global LAST_RESULT
    LAST_RESULT = res

    q01 = np.concatenate([res.results[i]["out"] for i in range(N_CORES)],
                         axis=1)
    act = np.clip(x[:, 4 * sel + 3].astype(np.int32), 0, 1)
    out = np.where(act == 0, q01[0], q01[1]).astype(np.float32)
    return out[:, None]


# revision 15
# speedup vs baseline: 2.8604x; 1.1683x over previous
"""Trainium2 Bass kernel for DecomposedQValueNN (gnn_message_passing).

Per batch row b of x[65536, 128]:
  xa = x.reshape(B, 32, 4); other_a = MLP_o(xa[:,a]) (3 relu layers, 4-32-32-16)
  sum_other = sum_{a != sel} other_a;  sel_out = MLP_s(xa[:,sel])
  h = relu([sel_out; sum_other] @ gW1 + gb1); q = h @ gW2 + gb2
  out[b] = q[b, clip(int(xa[b,sel,3]),0,1)]

V2 design (8 cores, batch data-parallel, 8192 rows/core):
  - host transposes + bf16-casts x to [feat=128, rows] so no on-device
    transpose is needed and DMA reads are contiguous per partition
  - all matmuls bf16 (single-pass on PE vs fp32's 2-pass); PSUM fp32
  - 32x32 PE-array tiling, loop order cycles row groups so LDWEIGHTS
    overlaps in-flight matmuls
  - PSUM pair-tiles [128,1024] (2 banks) so each PSUM->SBUF evacuation
    instruction covers 2 banks; evacs alternate ScalarE/VectorE
  - global head folded into the agent-sum: accumulating matmuls against
    replicated gW1_sum (sel agent excluded via a zeroed weight variant),
    plus one K=16 matmul adding gW1_sel^T @ sel_out
  Final 2-way q gather on host.
"""

import numpy as np
import ml_dtypes

BF16 = ml_dtypes.bfloat16

B_FULL = 65536
N_CORES = 8
B_C = B_FULL // N_CORES       # 8192
A, D = 32, 4
NCH = 512                     # batch cols per PSUM bank (fp32)
CHUNKS = B_C // NCH           # 16

# wpack (bf16) column offsets
OW1BD = 0          # [128, 256] L1 block-diag variants u=0..7
OW2 = 256          # [128, 32]
OW3P = 288         # [128, 32] K=64 2-agent pack: even row-grp W3->cols 0..15,
                   # odd row-grp W3->cols 16..31
OSW1 = 320         # [128, 32]
OSW2 = 352
OSW3 = 384
OGSUMF = 416       # [128, 32] gW1[16+(m%16)] at every partition 32j+m
OGSUME = 448       # same, sel agent's 16-row stripe zeroed
OGSEL = 480        # [16, 32] gW1[0:16]
OGW2 = 512         # [32, 32] gW2 padded
WCOLS = 544

_COMPILED = {}
LAST_RESULT = None


def _f32(a):
    return np.ascontiguousarray(a, dtype=np.float32)


def _build_wpack(sel, oW1, oW2, oW3, sW1, sW2, sW3, gW1, gW2):
    P = 128

    def rep4(w):              # [32, m] -> [128, m]
        return np.tile(_f32(w), (4, 1))

    def padc(w, m):
        w = _f32(w)
        return np.concatenate([w, np.zeros((w.shape[0], m - w.shape[1]), np.float32)], 1)

    def padr(w, m):
        w = _f32(w)
        return np.concatenate([w, np.zeros((m - w.shape[0], w.shape[1]), np.float32)], 0)

    # L1 weights as zero-padded K=32 blocks: block u has oW1 at rows 4u..4u+3
    w1bd = np.zeros((32, 8 * 32), np.float32)
    for u in range(8):
        w1bd[4 * u:4 * u + 4, 32 * u:32 * u + 32] = _f32(oW1)
    w1 = np.tile(w1bd, (4, 1))
    w2 = rep4(oW2)
    # K=64 2-agent pack: row-group r even -> z3 cols 0..15, odd -> 16..31
    w3L = padc(_f32(oW3), 32)                       # [32, 32] cols 0..15
    w3R = np.concatenate([np.zeros((32, 16), np.float32), _f32(oW3)], 1)
    w3 = np.concatenate([w3L, w3R, w3L, w3R], 0)    # [128, 32]
    u_sel = sel % 8
    sw1p = np.zeros((32, 32), np.float32)
    sw1p[4 * u_sel:4 * u_sel + 4, :] = _f32(sW1)
    sw1 = np.tile(sw1p, (4, 1))
    sw2 = rep4(_f32(sW2))
    sw3 = rep4(padc(sW3, 32))

    # z3s layout (K=64 packed): per p one pair-tile; half k=i1//2 holds
    # agents (8*i1+4p+j) at partitions 32j+0..15 and (8*(i1+1)+4p+j) at
    # 32j+16..31.  gsum weight: row 32j+m -> gW1[16+(m%16), :].
    g = _f32(gW1)
    gsumf = np.zeros((P, 32), np.float32)
    for j in range(4):
        gsumf[32 * j:32 * j + 16, :] = g[16:32, :]
        gsumf[32 * j + 16:32 * j + 32, :] = g[16:32, :]
    gsume = gsumf.copy()
    j_sel = sel % 4
    i_sel = sel // 8
    off = 32 * j_sel + 16 * (i_sel % 2)
    gsume[off:off + 16, :] = 0.0

    # gsel replicated: rows 32c+m (m<16) = gW1[m] (batched sel tile stripes)
    gselw = np.zeros((P, 32), np.float32)
    for c in range(4):
        gselw[32 * c:32 * c + 16, :] = g[0:16, :]
    gw2 = padr(padc(gW2, 32), P)         # wp[0:32, OGW2:]

    parts = [w1, w2, w3, sw1, sw2, sw3, gsumf, gsume, gselw, gw2]
    wp = np.concatenate(parts, axis=1)
    assert wp.shape == (P, WCOLS), wp.shape
    return np.ascontiguousarray(wp.astype(BF16))


def _build_bias(ob1, ob2, ob3, sb1, sb2, sb3, gb1, gb2):
    # fp32 per-partition bias columns: col k pattern = b[(p % 32) % len]
    P = 128

    def bias_col(b, valid=32):
        v = np.zeros((P, 1), np.float32)
        b = _f32(b).ravel()
        for p in range(P):
            r = p % 32
            if r < valid:
                v[p, 0] = b[r % len(b)]
        return v

    cols = [bias_col(ob1), bias_col(ob2), bias_col(ob3, 32),
            bias_col(sb1), bias_col(sb2), bias_col(sb3, 16),
            bias_col(gb1), bias_col(gb2, 2)]
    return np.ascontiguousarray(np.concatenate(cols, 1))  # [128, 8] fp32


BB1, BB2, BB3, BSB1, BSB2, BSB3, BGB1, BGB2 = range(8)


def _build_nc(sel):
    import concourse.bacc as bacc
    import concourse.mybir as mybir
    from concourse.tile import TileContext

    f32 = mybir.dt.float32
    bf16 = mybir.dt.bfloat16
    Relu = mybir.ActivationFunctionType.Relu
    Ident = mybir.ActivationFunctionType.Identity
    add_op = mybir.AluOpType.add
    max_op = mybir.AluOpType.max

    i_sel, j_sel = sel // 8, sel % 4
    p_sel = (sel % 8) // 4
    si = sel // 8          # partition group of sel agent's features

    nc = bacc.Bacc("TRN2", target_bir_lowering=False, debug=False,
                   num_devices=N_CORES)
    x_ext = nc.dram_tensor("xt", [128, B_C], bf16, kind="ExternalInput").ap()
    w_ext = nc.dram_tensor("wpack", [128, WCOLS], bf16, kind="ExternalInput").ap()
    b_ext = nc.dram_tensor("bias", [128, 8], f32, kind="ExternalInput").ap()
    o_ext = nc.dram_tensor("out", [2, B_C], f32, kind="ExternalOutput").ap()

    with TileContext(nc) as tc:
        with (
            tc.tile_pool(name="const", bufs=1) as cpool,
            tc.tile_pool(name="xin", bufs=6) as xpool,
            tc.tile_pool(name="h", bufs=5) as hpool,
            tc.tile_pool(name="z3s", bufs=11) as z3pool,
            tc.tile_pool(name="gl", bufs=3) as glpool,
            tc.tile_pool(name="osb", bufs=1) as opool,
            tc.tile_pool(name="zp", bufs=3, space="PSUM") as zpool,
            tc.tile_pool(name="gp", bufs=2, space="PSUM") as gpool,
        ):
            W = cpool.tile([128, WCOLS], bf16, name="W")
            nc.sync.dma_start(out=W[:], in_=w_ext[:])
            BI = cpool.tile([128, 8], f32, name="BI")
            nc.sync.dma_start(out=BI[:], in_=b_ext[:])
            outsb = opool.tile([2, B_C], f32, name="outsb")

            def evac(dst, src, bcol, func, dve, lo=0, size=128):
                b = BI[lo:lo + size, bcol:bcol + 1]
                if dve:
                    if func == "relu":
                        nc.vector.tensor_scalar(dst, src, b, 0.0, add_op, max_op)
                    else:
                        nc.vector.tensor_scalar_add(dst, src, b)
                else:
                    if func == "relu":
                        nc.scalar.activation(dst, src, Relu, bias=b)
                    else:
                        nc.scalar.activation(dst, src, Ident, bias=b)

            xin_t = {}
            z3s_all = {}
            bsh3_g = {}
            tail_q = None

            def emit_main_L1(c):
                b0 = c * NCH
                xin = xpool.tile([128, NCH], bf16, tag="xin", name=f"xin{c}")
                nc.sync.dma_start(out=xin[:], in_=x_ext[:, b0:b0 + NCH])
                xin_t[c] = xin
                h1s = {}
                for p in range(2):
                    z1 = [zpool.tile([128, 2 * NCH], f32, tag="z",
                                     name=f"z1_{c}_{p}_{k}") for k in range(2)]
                    for j in range(4):
                        for i in range(4):
                            u = 4 * p + j
                            nc.tensor.matmul(
                                z1[i // 2][32 * j:32 * j + 32,
                                           NCH * (i % 2):NCH * (i % 2) + NCH],
                                W[32 * i:32 * i + 32,
                                  OW1BD + 32 * u:OW1BD + 32 * u + 32],
                                xin[32 * i:32 * i + 32, :],
                                start=True, stop=True,
                                tile_position=(32 * i, 32 * j))
                    for k in range(2):
                        h1 = hpool.tile([128, 2 * NCH], bf16, tag="h1",
                                        name=f"h1_{c}_{p}_{k}")
                        evac(h1[:], z1[k][:], BB1, "relu", dve=(k == 0))
                        h1s[(p, k)] = h1
                return h1s

            def emit_main_L2(c, h1s):
                h2s = {}
                for p in range(2):
                    z2 = [zpool.tile([128, 2 * NCH], f32, tag="z",
                                     name=f"z2_{c}_{p}_{k}") for k in range(2)]
                    for i in range(4):
                        for j in range(4):
                            nc.tensor.matmul(
                                z2[j // 2][32 * i:32 * i + 32,
                                           NCH * (j % 2):NCH * (j % 2) + NCH],
                                W[32 * j:32 * j + 32, OW2:OW2 + 32],
                                h1s[(p, i // 2)][32 * j:32 * j + 32,
                                                 NCH * (i % 2):NCH * (i % 2) + NCH],
                                start=True, stop=True,
                                tile_position=(32 * j, 32 * i))
                    for k in range(2):
                        h2 = hpool.tile([128, 2 * NCH], bf16, tag="h2",
                                        name=f"h2_{c}_{p}_{k}")
                        evac(h2[:], z2[k][:], BB2, "relu", dve=(k == 0))
                        h2s[(p, k)] = h2
                return h2s

            def emit_main_L3(c, h2s):
                z3s = {}
                for p in range(2):
                    z3 = zpool.tile([128, 2 * NCH], f32, tag="z",
                                    name=f"z3_{c}_{p}")
                    for j in range(4):
                        for i1 in (0, 2):
                            nc.tensor.matmul(
                                z3[32 * j:32 * j + 32,
                                   NCH * (i1 // 2):NCH * (i1 // 2) + NCH],
                                W[32 * i1:32 * i1 + 64, OW3P:OW3P + 32],
                                h2s[(p, j // 2)][32 * i1:32 * i1 + 64,
                                                 NCH * (j % 2):NCH * (j % 2) + NCH],
                                start=True, stop=True,
                                tile_position=(32 * i1, 32 * j))
                    z3sb = z3pool.tile([128, 2 * NCH], bf16, tag="z3s",
                                       name=f"z3s_{c}_{p}")
                    evac(z3sb[:], z3[:], BB3, "relu", dve=False)
                    z3s[p] = z3sb
                z3s_all[c] = z3s

            def emit_selb(grp):
                # batched sel-MLP for chunks 4g..4g+3: chunk stripe = 32*(c%4)
                bz1 = gpool.tile([128, NCH], f32, tag="g", name=f"bz1_{grp}")
                for cl in range(4):
                    cc = 4 * grp + cl
                    nc.tensor.matmul(
                        bz1[32 * cl:32 * cl + 32, :],
                        W[32 * si:32 * si + 32, OSW1:OSW1 + 32],
                        xin_t[cc][32 * si:32 * si + 32, :],
                        start=True, stop=True,
                        tile_position=(32 * si, 32 * cl))
                bsh1 = glpool.tile([128, NCH], bf16, tag="sh1", name=f"bsh1_{grp}")
                evac(bsh1[:], bz1[:], BSB1, "relu", dve=True)
                bz2 = gpool.tile([128, NCH], f32, tag="g", name=f"bz2_{grp}")
                for cl in range(4):
                    nc.tensor.matmul(
                        bz2[32 * cl:32 * cl + 32, :],
                        W[32 * cl:32 * cl + 32, OSW2:OSW2 + 32],
                        bsh1[32 * cl:32 * cl + 32, :],
                        start=True, stop=True,
                        tile_position=(32 * cl, 32 * cl))
                bsh2 = glpool.tile([128, NCH], bf16, tag="sh2", name=f"bsh2_{grp}")
                evac(bsh2[:], bz2[:], BSB2, "relu", dve=True)
                bz3 = gpool.tile([128, NCH], f32, tag="g", name=f"bz3_{grp}")
                for cl in range(4):
                    nc.tensor.matmul(
                        bz3[32 * cl:32 * cl + 32, :],
                        W[32 * cl:32 * cl + 32, OSW3:OSW3 + 32],
                        bsh2[32 * cl:32 * cl + 32, :],
                        start=True, stop=True,
                        tile_position=(32 * cl, 32 * cl))
                bsh3 = glpool.tile([128, NCH], bf16, tag="sh3", name=f"bsh3_{grp}")
                evac(bsh3[:], bz3[:], BSB3, "relu", dve=True)
                bsh3_g[grp] = bsh3

            def emit_tail_A(c):
                # first 3 accumulating gsum matmuls
                zg = gpool.tile([32, NCH], f32, tag="g", name=f"zg_{c}")
                z3s = z3s_all[c]
                n_mm = 0
                for p in range(2):
                    for k in range(2):
                        if n_mm >= 3:
                            break
                        excl = (p == p_sel and k == i_sel // 2)
                        oo = OGSUME if excl else OGSUMF
                        nc.tensor.matmul(
                            zg[:], W[:, oo:oo + 32],
                            z3s[p][:, NCH * k:NCH * k + NCH],
                            start=(n_mm == 0), stop=False)
                        n_mm += 1
                return zg

            def emit_tail_B(c, zg):
                z3s = z3s_all[c]
                excl = (1 == p_sel and 1 == i_sel // 2)
                oo = OGSUME if excl else OGSUMF
                nc.tensor.matmul(
                    zg[:], W[:, oo:oo + 32],
                    z3s[1][:, NCH:NCH + NCH],
                    start=False, stop=False)
                cl = c % 4
                bsh3 = bsh3_g[c // 4]
                nc.tensor.matmul(
                    zg[:], W[32 * cl:32 * cl + 16, OGSEL:OGSEL + 32],
                    bsh3[32 * cl:32 * cl + 16, :],
                    start=False, stop=True, tile_position=(32 * cl, 0))

            def emit_tail_C(c, zg):
                b0 = c * NCH
                hg = glpool.tile([32, NCH], bf16, tag="hg", name=f"hg_{c}")
                evac(hg[:], zg[:], BGB1, "relu", dve=True, size=32)
                qp = gpool.tile([32, NCH], f32, tag="g", name=f"qp_{c}")
                nc.tensor.matmul(
                    qp[:], W[0:32, OGW2:OGW2 + 32], hg[0:32, :],
                    start=True, stop=True, tile_position=(0, 0))
                evac(outsb[0:2, b0:b0 + NCH], qp[0:2, :], BGB2, "add",
                     dve=False, size=2)
                del z3s_all[c]

            LAG = 4  # tail(c) needs bsh3 of group c//4 (ready after chunk 4g+3)
            for c in range(CHUNKS + LAG):
                tail = c - LAG if c >= LAG else None
                if c < CHUNKS:
                    h1s = emit_main_L1(c)
                if tail is not None:
                    zg = emit_tail_A(tail)
                if c < CHUNKS:
                    h2s = emit_main_L2(c, h1s)
                if tail is not None:
                    emit_tail_B(tail, zg)
                if c < CHUNKS:
                    emit_main_L3(c, h2s)
                if tail is not None:
                    emit_tail_C(tail, zg)
                if c < CHUNKS and c % 4 == 3:
                    emit_selb(c // 4)

            nc.sync.dma_start(out=o_ext[:], in_=outsb[:])
    nc.compile()
    return nc


def kernel(**inputs):
    x = _f32(inputs["joint_state_actions"])
    sel = int(inputs["selected_agent_idx"])

    wpack = _build_wpack(
        sel, inputs["oW1"], inputs["oW2"], inputs["oW3"],
        inputs["sW1"], inputs["sW2"], inputs["sW3"],
        inputs["gW1"], inputs["gW2"])
    bias = _build_bias(
        inputs["ob1"], inputs["ob2"], inputs["ob3"],
        inputs["sb1"], inputs["sb2"], inputs["sb3"],
        inputs["gb1"], inputs["gb2"])

    if sel not in _COMPILED:
        _COMPILED[sel] = _build_nc(sel)
    nc = _COMPILED[sel]

    from concourse.bass_utils import run_bass_kernel_spmd
    shards = [np.ascontiguousarray(x[i * B_C:(i + 1) * B_C].T.astype(BF16))
              for i in range(N_CORES)]
    in_maps = [{"xt": s, "wpack": wpack, "bias": bias} for s in shards]
    import os
    trace = bool(int(os.environ.get("KERNEL_TRACE", "0")))
    res = run_bass_kernel_spmd(nc, in_maps, list(range(N_CORES)),
                               trace=trace)
    global LAST_RESULT
    LAST_RESULT = res

    q01 = np.concatenate([res.results[i]["out"] for i in range(N_CORES)],
                         axis=1)
    act = np.clip(x[:, 4 * sel + 3].astype(np.int32), 0, 1)
    out = np.where(act == 0, q01[0], q01[1]).astype(np.float32)
    return out[:, None]
